# revision 2
# baseline (speedup 1.0000x reference)
import numpy as np
from contextlib import ExitStack

import concourse.bass as bass
import concourse.tile as tile
from concourse import bacc, mybir

# problem constants (hardcoded per contract)
N = 4096          # points
C = 20            # feature channels
K = 6             # boxes
M = 3             # views
G = K * M         # 18 groups
RES = 48          # H = W
NCORES = 8
SROWS = RES // NCORES          # 6 grid rows per core
SLOC = SROWS * RES             # 288 cells per core
NBLK = N // 128                # 32 point blocks
NSAMPLE = 16
RADIUS2 = 9.0

TRACE = False
_last = {}

_f32 = mybir.dt.float32
_ALU = mybir.AluOpType
_ACT = mybir.ActivationFunctionType


def _build_nc():
    nc = bacc.Bacc("TRN2", target_bir_lowering=False, debug=False, num_devices=NCORES)
    A = nc.dram_tensor("A", [G, 4, N], _f32, kind="ExternalInput").ap()
    P = nc.dram_tensor("P", [128, 4 * NBLK], _f32, kind="ExternalInput").ap()
    TRI = nc.dram_tensor("TRI", [128, 128], _f32, kind="ExternalInput").ap()
    IEYE = nc.dram_tensor("IEYE", [4, 4], _f32, kind="ExternalInput").ap()
    CAR4 = nc.dram_tensor("CAR4", [4, 128], _f32, kind="ExternalInput").ap()
    B4 = nc.dram_tensor("B4", [4, SLOC], _f32, kind="ExternalInput").ap()
    OUT = nc.dram_tensor("OUT", [G, SLOC], _f32, kind="ExternalOutput").ap()

    with ExitStack() as ctx:
        tc = ctx.enter_context(tile.TileContext(nc))
        consts = ctx.enter_context(tc.tile_pool(name="consts", bufs=1))
        apool = ctx.enter_context(tc.tile_pool(name="apool", bufs=2))
        wpool = ctx.enter_context(tc.tile_pool(name="wpool", bufs=4))
        spool = ctx.enter_context(tc.tile_pool(name="spool", bufs=4))
        rowpool = ctx.enter_context(tc.tile_pool(name="rowpool", bufs=4))
        fpool = ctx.enter_context(tc.tile_pool(name="fpool", bufs=4))
        pspool = ctx.enter_context(
            tc.tile_pool(name="ps", bufs=2, space=bass.MemorySpace.PSUM))
        agpool = ctx.enter_context(
            tc.tile_pool(name="agps", bufs=2, space=bass.MemorySpace.PSUM))

        p_t = consts.tile([128, 4 * NBLK], _f32)
        nc.sync.dma_start(p_t[:], P)
        tri_t = consts.tile([128, 128], _f32)
        nc.sync.dma_start(tri_t[:], TRI)
        ieye_t = consts.tile([4, 4], _f32)
        nc.sync.dma_start(ieye_t[:], IEYE)
        b4_t = consts.tile([4, SLOC], _f32)
        nc.sync.dma_start(b4_t[:], B4)
        car4_t = consts.tile([4, 128], _f32)
        nc.sync.dma_start(car4_t[:], CAR4)
        w4_t = consts.tile([128, 4], _f32)
        nc.vector.memset(w4_t[:, 0:1], 1.0)
        nc.vector.memset(w4_t[:, 1:4], 0.0)

        for g in range(G):
            a_t = apool.tile([4, N], _f32)
            nc.sync.dma_start(a_t[:], A[g])
            state_sb = None
            for b in range(NBLK):
                score_ps = pspool.tile([128, SLOC], _f32)
                nc.tensor.matmul(score_ps[:], a_t[:, 128 * b:128 * (b + 1)],
                                 b4_t[:], start=True, stop=True)
                within = wpool.tile([128, SLOC], _f32)
                nc.vector.tensor_scalar(within[:], score_ps[:], 0.0, None,
                                        _ALU.is_gt)
                # u = carry(prev blocks) + excl_prefix - 16*within
                u_ps = pspool.tile([128, SLOC], _f32)
                nc.tensor.matmul(u_ps[:], tri_t[:], within[:],
                                 start=True, stop=(b == 0))
                if b > 0:
                    nc.tensor.matmul(u_ps[:], car4_t[:], state_sb[:],
                                     start=False, stop=True)
                sel = spool.tile([128, SLOC], _f32)
                nc.vector.tensor_scalar(sel[:], u_ps[:], 0.0, None, _ALU.is_lt)
                # state rows: [carry, s0, s1, cnt]
                state_ps = agpool.tile([4, SLOC], _f32)
                nc.tensor.matmul(state_ps[:], p_t[:, 4 * b:4 * (b + 1)],
                                 sel[:], start=True, stop=False)
                nc.tensor.matmul(state_ps[:], w4_t[:], within[:],
                                 start=False, stop=(b == 0))
                if b > 0:
                    nc.tensor.matmul(state_ps[:], ieye_t[:], state_sb[:],
                                     start=False, stop=True)
                state_sb = rowpool.tile([4, SLOC], _f32)
                nc.scalar.activation(state_sb[:], state_ps[:], _ACT.Copy)

            # finalize: p1 = (cnt>0) * sigmoid((s1-s0)/max(cnt,1)) * 255
            s0_t = fpool.tile([1, SLOC], _f32, tag="s0")
            s1_t = fpool.tile([1, SLOC], _f32, tag="s1")
            cnt_t = fpool.tile([1, SLOC], _f32, tag="cnt")
            nc.sync.dma_start(s0_t[:], state_sb[1:2, :])
            nc.sync.dma_start(s1_t[:], state_sb[2:3, :])
            nc.sync.dma_start(cnt_t[:], state_sb[3:4, :])
            cntc = fpool.tile([1, SLOC], _f32, tag="cntc")
            nc.vector.tensor_scalar(cntc[:], cnt_t[:], 1.0, None,
                                    _ALU.max)
            rcp = fpool.tile([1, SLOC], _f32, tag="rcp")
            nc.vector.reciprocal(rcp[:], cntc[:])
            dd = fpool.tile([1, SLOC], _f32, tag="dd")
            nc.vector.tensor_tensor(dd[:], s1_t[:], s0_t[:],
                                    _ALU.subtract)
            nfd = fpool.tile([1, SLOC], _f32, tag="nfd")
            nc.vector.tensor_tensor(nfd[:], dd[:], rcp[:], _ALU.mult)
            sig = fpool.tile([1, SLOC], _f32, tag="sig")
            nc.scalar.activation(sig[:], nfd[:], _ACT.Sigmoid)
            gate = fpool.tile([1, SLOC], _f32, tag="gate")
            nc.vector.tensor_scalar(gate[:], cnt_t[:], 0.5, 255.0,
                                    _ALU.is_gt, _ALU.mult)
            orow = fpool.tile([1, SLOC], _f32, tag="orow")
            nc.vector.tensor_tensor(orow[:], sig[:], gate[:], _ALU.mult)
            nc.sync.dma_start(OUT[g:g + 1, :], orow[:])
    nc.compile()
    return nc


# ---------------------------------------------------------------------------
# Cached SPMD dispatch. run_bass_kernel_spmd rebuilds its jax.jit closure on
# every invocation, which forces a full XLA retrace+recompile (~0.9s) per
# call; the NEFF itself is unchanged between calls. Build the jitted
# shard_map executable once and reuse it, and fetch the output with a single
# host transfer.
# ---------------------------------------------------------------------------

_DISP = None


def _build_dispatch(nc):
    import jax
    from jax.experimental.shard_map import shard_map
    from jax.sharding import Mesh, NamedSharding, PartitionSpec
    from concourse.bass2jax import (
        _bass_exec_p, install_neuronx_cc_hook, partition_id_tensor)

    install_neuronx_cc_hook()
    assert nc.dbg_addr is None

    partition_name = nc.partition_id_tensor.name if nc.partition_id_tensor else None
    in_names, out_names, out_avals, zero_shapes = [], [], [], []
    for alloc in nc.m.functions[0].allocations:
        if not isinstance(alloc, mybir.MemoryLocationSet):
            continue
        name = alloc.memorylocations[0].name
        if alloc.kind == "ExternalInput":
            if name != partition_name:
                in_names.append(name)
        elif alloc.kind == "ExternalOutput":
            shape = tuple(alloc.tensor_shape)
            dtype = mybir.dt.np(alloc.dtype)
            out_names.append(name)
            out_avals.append(jax.core.ShapedArray(shape, dtype))
            zero_shapes.append((shape, dtype))
    n_params = len(in_names)
    bind_names = in_names + out_names
    if partition_name is not None:
        bind_names.append(partition_name)

    def _body(*args):
        operands = list(args)
        if partition_name is not None:
            operands.append(partition_id_tensor())
        outs = _bass_exec_p.bind(
            *operands,
            out_avals=tuple(out_avals),
            in_names=tuple(bind_names),
            out_names=tuple(out_names),
            lowering_input_output_aliases=(),
            sim_require_finite=True,
            sim_require_nnan=True,
            nc=nc,
        )
        return tuple(outs)

    devices = jax.devices()[:NCORES]
    assert len(devices) == NCORES
    mesh = Mesh(np.asarray(devices), ("core",))
    n_outs = len(out_names)
    donate = tuple(range(n_params, n_params + n_outs))
    in_specs = (PartitionSpec("core"),) * (n_params + n_outs)
    out_specs = (PartitionSpec("core"),) * n_outs
    sharded = jax.jit(
        shard_map(_body, mesh=mesh, in_specs=in_specs, out_specs=out_specs,
                  check_rep=False),
        donate_argnums=donate,
        keep_unused=True,
    )
    sharding = NamedSharding(mesh, PartitionSpec("core"))

    def put(x):
        """Commit a per-call-constant global array to the devices once."""
        return jax.device_put(x, sharding)

    return {
        "fn": sharded, "in_names": in_names, "out_names": out_names,
        "zero_shapes": zero_shapes, "put": put,
    }


def _dispatch(nc, in_maps):
    global _DISP
    if _DISP is None:
        _DISP = _build_dispatch(nc)
    d = _DISP
    # constants (value-independent inputs) are committed to device once
    if "const_cache" not in d:
        d["const_cache"] = {}
    concat_in = []
    for name in d["in_names"]:
        if name in d["const_cache"]:
            concat_in.append(d["const_cache"][name])
            continue
        arr = np.concatenate([np.asarray(m[name]) for m in in_maps], axis=0)
        concat_in.append(arr)
    concat_zeros = [np.zeros((NCORES * s[0], *s[1:]), dt)
                    for s, dt in d["zero_shapes"]]
    out_arrs = d["fn"](*concat_in, *concat_zeros)
    # single host fetch per output
    return {name: np.asarray(out_arrs[i]) for i, name in enumerate(d["out_names"])}


def _mark_const(names, in_maps):
    """Commit value-independent inputs to device memory once; later calls
    skip the host->device transfer for them."""
    d = _DISP
    for name in names:
        if name not in d["const_cache"]:
            arr = np.concatenate([np.asarray(m[name]) for m in in_maps], axis=0)
            d["const_cache"][name] = d["put"](arr)


_nc_cache = None


def kernel(xyz, features, boxes, theta, phi, res):
    global _nc_cache
    xyz = np.asarray(xyz, np.float32)[0]        # (N,3)
    features = np.asarray(features, np.float32)[0]  # (N,C)
    boxes = np.asarray(boxes, np.float32)[0]    # (K,6)
    theta = np.asarray(theta, np.float32)
    phi = np.asarray(phi, np.float32)
    res = int(res)
    H = W = res

    # ---- host prep: O(N*K + N*C) index/constant prep; heavy O(G*S*N) on device
    sint, cost = np.sin(theta), np.cos(theta)
    sinp, cosp = np.sin(phi), np.cos(phi)
    U = np.stack([-sint, cost, np.zeros_like(theta)], -1)
    V = np.stack([cost * sinp, sint * sinp, cosp], -1)
    basis = np.stack([U, V], -1).astype(np.float32)          # (M,3,2)
    center3 = np.stack([cost * cosp, sint * cosp, sinp], -1).astype(np.float32)
    coords_mv = np.einsum('mnd,mdk->mnk',
                          (xyz[None] - center3[:, None]).astype(np.float32),
                          basis).astype(np.float32)          # (M,N,2)
    valid = (np.all(xyz[None] <= boxes[:, None, 3:], -1)
             & np.all(xyz[None] >= boxes[:, None, :3], -1))  # (K,N)
    pts = np.sort(features, -1)[:, -2:].astype(np.float32)   # (N,2)
    p2 = np.array([H, W], np.float32)

    A = np.zeros((G, 4, N), np.float32)
    for k in range(K):
        vmask = valid[k]
        for m in range(M):
            c = coords_mv[m].copy()
            vc = c[vmask]
            cmin = vc.min(0)
            cmax = vc.max(0)
            ctr = ((cmax + cmin) / 2).astype(np.float32)
            scale = (np.maximum(cmax - cmin, np.float32(1e-5)) / 2).astype(np.float32)
            cn = (((c - ctr) / scale + np.float32(1.0)) * np.float32(0.8)
                  * p2 / 2 + np.float32(0.1) * p2).astype(np.float32)
            cn[~vmask] = 1e6
            g = k * M + m
            A[g, 0] = cn[:, 0]
            A[g, 1] = cn[:, 1]
            A[g, 2] = cn[:, 0] * cn[:, 0] + cn[:, 1] * cn[:, 1]
            A[g, 3] = 1.0
    P = np.concatenate([np.zeros((N, 1), np.float32), pts,
                        np.ones((N, 1), np.float32)], 1)  # (N,4)
    P = P.reshape(NBLK, 128, 4).transpose(1, 0, 2).reshape(128, 4 * NBLK).copy()
    TRI = np.triu(np.ones((128, 128), np.float32), 1)
    np.fill_diagonal(TRI, -float(NSAMPLE))
    IEYE = np.eye(4, dtype=np.float32)
    CAR4 = np.zeros((4, 128), np.float32)
    CAR4[0] = 1.0

    gx, gy = np.meshgrid(np.arange(H), np.arange(W), indexing='ij')
    samples = np.stack([gx, gy], -1).reshape(-1, 2).astype(np.float32)  # (S,2)
    in_maps = []
    for cidx in range(NCORES):
        s = samples[cidx * SLOC:(cidx + 1) * SLOC]
        B4 = np.stack([
            2.0 * s[:, 0], 2.0 * s[:, 1],
            -np.ones(SLOC, np.float32),
            RADIUS2 - (s[:, 0] ** 2 + s[:, 1] ** 2),
        ]).astype(np.float32)
        in_maps.append({"A": A, "P": P, "TRI": TRI, "IEYE": IEYE, "CAR4": CAR4, "B4": B4})

    if _nc_cache is None:
        _nc_cache = _build_nc()
    results = _dispatch(_nc_cache, in_maps)
    # TRI/IEYE/CAR4/B4 do not depend on input values: commit to device once
    _mark_const(["TRI", "IEYE", "CAR4", "B4"], in_maps)
    _last['exec_time_ns'] = None
    out_g = results["OUT"].reshape(NCORES, G, SROWS, W)
    full = np.concatenate([out_g[c] for c in range(NCORES)], axis=1)  # (G,H,W)
    out = np.broadcast_to(full[:, None, :, :], (G, 3, H, W)).astype(np.float32)
    return np.ascontiguousarray(out)


# revision 13
# speedup vs baseline: 6.5360x; 6.5360x over previous
import numpy as np
from contextlib import ExitStack

import concourse.bass as bass
import concourse.tile as tile
from concourse import bacc, mybir

# problem constants (hardcoded per contract)
N = 4096          # points
C = 20            # feature channels
K = 6             # boxes
M = 3             # views
G = K * M         # 18 groups
RES = 48          # H = W
NCORES = 8
SROWS = RES // NCORES          # 6 grid rows per core
SLOC = SROWS * RES             # 288 cells per core
NSAMPLE = 16
RADIUS2 = 9.0

TRACE = False
_last = {}

_f32 = mybir.dt.float32
_bf16 = mybir.dt.bfloat16
_ALU = mybir.AluOpType
_ACT = mybir.ActivationFunctionType
BF16 = mybir.dt.np(_bf16)

# x-slab half-width: a point can only be in-ball for a cell row gx when
# |x - gx| < RADIUS, so core c (rows 6c..6c+5) only needs x in (6c-3, 6c+8)
SLAB_LO = -3.0
SLAB_HI = float(SROWS) + 5.0


def _build_nc(capblks):
    """Per-group-capped ball query + first-16 aggregation.

    Inputs per core (slab-filtered points, group-major):
      AX  [3, 128*sum(capblks)] f32  rows [x, y, x^2+y^2] (pad: 1e6,1e6,2e12)
      PD  [128, 2*sum(capblks)] bf16 per-block stationary cols [s1-s0, 1]
      B4  [4, SLOC] f32  cell polynomials [2sx, 2sy, -1, R^2-sx^2-sy^2]
      TRIB/FIXB [128,128] bf16  prefix-scan / full-sum upgrade matrices
    Output: OUT [G, SLOC] f32.
    """
    BTOT = int(sum(capblks))
    PTOT = 128 * BTOT
    nc = bacc.Bacc("TRN2", target_bir_lowering=False, debug=False,
                   num_devices=NCORES)
    AX = nc.dram_tensor("AX", [4, PTOT], _f32, kind="ExternalInput").ap()
    PD = nc.dram_tensor("PD", [128, 2 * BTOT], _bf16, kind="ExternalInput").ap()
    B4 = nc.dram_tensor("B4", [4, SLOC], _f32, kind="ExternalInput").ap()
    TRIB = nc.dram_tensor("TRIB", [128, 128], _bf16, kind="ExternalInput").ap()
    FIXB = nc.dram_tensor("FIXB", [128, 128], _bf16, kind="ExternalInput").ap()
    OUT = nc.dram_tensor("OUT", [G, SLOC], _f32, kind="ExternalOutput").ap()

    with ExitStack() as ctx:
        tc = ctx.enter_context(tile.TileContext(nc))
        consts = ctx.enter_context(tc.tile_pool(name="consts", bufs=1))
        apool = ctx.enter_context(tc.tile_pool(name="apool", bufs=2))
        ppool = ctx.enter_context(tc.tile_pool(name="ppool", bufs=2))
        wpool = ctx.enter_context(tc.tile_pool(name="wpool", bufs=4))
        spool = ctx.enter_context(tc.tile_pool(name="spool", bufs=4))
        fin = ctx.enter_context(tc.tile_pool(name="fin", bufs=1))
        rowpool = ctx.enter_context(tc.tile_pool(name="rowpool", bufs=2))
        scps = ctx.enter_context(
            tc.tile_pool(name="scps", bufs=3, space=bass.MemorySpace.PSUM))
        ups = ctx.enter_context(
            tc.tile_pool(name="ups", bufs=2, space=bass.MemorySpace.PSUM))
        stps = ctx.enter_context(
            tc.tile_pool(name="stps", bufs=2, space=bass.MemorySpace.PSUM))

        b4_t = consts.tile([4, SLOC], _f32)
        nc.sync.dma_start(b4_t[:], B4)
        tri_t = consts.tile([128, 128], _bf16)
        nc.sync.dma_start(tri_t[:], TRIB)
        fix_t = consts.tile([128, 128], _bf16)
        nc.sync.dma_start(fix_t[:], FIXB)

        d_all = fin.tile([G, SLOC], _f32, tag="d_all")
        c_all = fin.tile([G, SLOC], _f32, tag="c_all")

        poff = 0
        boff = 0
        for g in range(G):
            nb = int(capblks[g])
            a_t = apool.tile([4, 128 * nb], _f32, tag="a")
            nc.sync.dma_start(a_t[:], AX[:, poff:poff + 128 * nb])
            p_t = ppool.tile([128, 2 * nb], _bf16, tag="p")
            nc.sync.dma_start(p_t[:], PD[:, 2 * boff:2 * (boff + nb)])
            u_ps = ups.tile([128, SLOC], _f32, tag="u")
            st_ps = stps.tile([2, SLOC], _f32, tag="st")
            for b in range(nb):
                score_ps = scps.tile([128, SLOC], _f32, tag="sc")
                nc.tensor.matmul(score_ps[:], a_t[:, 128 * b:128 * (b + 1)],
                                 b4_t[:], start=True, stop=True)
                within = wpool.tile([128, SLOC], _bf16, tag="w")
                nc.vector.tensor_scalar(within[:], score_ps[:], 0.0, None,
                                        _ALU.is_gt)
                # u = (total within count of prior blocks) + excl prefix
                #     - 16*within; sel = u < 0 picks the first 16 in-ball
                nc.tensor.matmul(u_ps[:], tri_t[:], within[:],
                                 start=(b == 0), stop=(b == nb - 1))
                sel = spool.tile([128, SLOC], _bf16, tag="s")
                nc.vector.tensor_scalar(sel[:], u_ps[:], 0.0, None, _ALU.is_lt)
                if b < nb - 1:
                    # upgrade this block's TRI contribution to its full
                    # within-count so u carries across blocks
                    nc.tensor.matmul(u_ps[:], fix_t[:], within[:],
                                     start=False, stop=False)
                # state rows: [sum(s1-s0), cnt] over selected points
                nc.tensor.matmul(st_ps[:], p_t[:, 2 * b:2 * (b + 1)], sel[:],
                                 start=(b == 0), stop=(b == nb - 1))
            # compute engines need 32-aligned partition bases; bounce the
            # state rows through SBUF and scatter with DMA (no such limit)
            tmp = rowpool.tile([2, SLOC], _f32, tag="tmp")
            nc.scalar.activation(tmp[:], st_ps[:], _ACT.Copy)
            nc.sync.dma_start(d_all[g:g + 1, :], tmp[0:1, :])
            nc.sync.dma_start(c_all[g:g + 1, :], tmp[1:2, :])
            poff += 128 * nb
            boff += nb

        # finalize all groups at once:
        # out = (cnt>0) * sigmoid(sum(s1-s0)/max(cnt,1)) * 255
        cntc = fin.tile([G, SLOC], _f32, tag="cntc")
        nc.vector.tensor_scalar(cntc[:], c_all[:], 1.0, None, _ALU.max)
        rcp = fin.tile([G, SLOC], _f32, tag="rcp")
        nc.vector.reciprocal(rcp[:], cntc[:])
        nfd = fin.tile([G, SLOC], _f32, tag="nfd")
        nc.vector.tensor_tensor(nfd[:], d_all[:], rcp[:], _ALU.mult)
        sig = fin.tile([G, SLOC], _f32, tag="sig")
        nc.scalar.activation(sig[:], nfd[:], _ACT.Sigmoid)
        gate = fin.tile([G, SLOC], _f32, tag="gate")
        nc.vector.tensor_scalar(gate[:], c_all[:], 0.5, 255.0,
                                _ALU.is_gt, _ALU.mult)
        orow = fin.tile([G, SLOC], _f32, tag="orow")
        nc.vector.tensor_tensor(orow[:], sig[:], gate[:], _ALU.mult)
        nc.sync.dma_start(OUT, orow[:])
    nc.compile()
    return nc


# ---------------------------------------------------------------------------
# Cached SPMD dispatch. run_bass_kernel_spmd rebuilds its jax.jit closure on
# every invocation, which forces a full XLA retrace+recompile (~0.9s) per
# call; the NEFF itself is unchanged between calls. Build the jitted
# shard_map executable once and reuse it, fetch the output with a single
# host transfer, and keep value-independent inputs resident on device.
# ---------------------------------------------------------------------------

_CACHE = {}
_CONST_NAMES = {"B4", "TRIB", "FIXB"}


def _build_dispatch(nc):
    import jax
    from jax.experimental.shard_map import shard_map
    from jax.sharding import Mesh, NamedSharding, PartitionSpec
    from concourse.bass2jax import (
        _bass_exec_p, install_neuronx_cc_hook, partition_id_tensor)

    install_neuronx_cc_hook()
    assert nc.dbg_addr is None

    partition_name = nc.partition_id_tensor.name if nc.partition_id_tensor else None
    in_names, out_names, out_avals, zero_shapes = [], [], [], []
    for alloc in nc.m.functions[0].allocations:
        if not isinstance(alloc, mybir.MemoryLocationSet):
            continue
        name = alloc.memorylocations[0].name
        if alloc.kind == "ExternalInput":
            if name != partition_name:
                in_names.append(name)
        elif alloc.kind == "ExternalOutput":
            shape = tuple(alloc.tensor_shape)
            dtype = mybir.dt.np(alloc.dtype)
            out_names.append(name)
            out_avals.append(jax.core.ShapedArray(shape, dtype))
            zero_shapes.append((shape, dtype))
    n_params = len(in_names)
    bind_names = in_names + out_names
    if partition_name is not None:
        bind_names.append(partition_name)

    def _body(*args):
        operands = list(args)
        if partition_name is not None:
            operands.append(partition_id_tensor())
        outs = _bass_exec_p.bind(
            *operands,
            out_avals=tuple(out_avals),
            in_names=tuple(bind_names),
            out_names=tuple(out_names),
            lowering_input_output_aliases=(),
            sim_require_finite=True,
            sim_require_nnan=True,
            nc=nc,
        )
        return tuple(outs)

    devices = jax.devices()[:NCORES]
    assert len(devices) == NCORES
    mesh = Mesh(np.asarray(devices), ("core",))
    n_outs = len(out_names)
    donate = tuple(range(n_params, n_params + n_outs))
    in_specs = (PartitionSpec("core"),) * (n_params + n_outs)
    out_specs = (PartitionSpec("core"),) * n_outs
    sharded = jax.jit(
        shard_map(_body, mesh=mesh, in_specs=in_specs, out_specs=out_specs,
                  check_rep=False),
        donate_argnums=donate,
        keep_unused=True,
    )
    sharding = NamedSharding(mesh, PartitionSpec("core"))

    def put(x):
        return jax.device_put(x, sharding)

    return {
        "fn": sharded, "in_names": in_names, "out_names": out_names,
        "zero_shapes": zero_shapes, "put": put,
    }


def _dispatch(in_maps):
    d = _CACHE["disp"]
    if "const_cache" not in d:
        d["const_cache"] = {
            name: d["put"](
                np.concatenate([np.asarray(m[name]) for m in in_maps], axis=0))
            for name in d["in_names"] if name in _CONST_NAMES
        }
    concat_in = []
    for name in d["in_names"]:
        if name in d["const_cache"]:
            concat_in.append(d["const_cache"][name])
            continue
        arr = np.concatenate([np.asarray(m[name]) for m in in_maps], axis=0)
        concat_in.append(arr)
    concat_zeros = [np.zeros((NCORES * s[0], *s[1:]), dt)
                    for s, dt in d["zero_shapes"]]
    out_arrs = d["fn"](*concat_in, *concat_zeros)
    return {name: np.asarray(out_arrs[i]) for i, name in enumerate(d["out_names"])}


def kernel(xyz, features, boxes, theta, phi, res):
    xyz = np.asarray(xyz, np.float32)[0]        # (N,3)
    features = np.asarray(features, np.float32)[0]  # (N,C)
    boxes = np.asarray(boxes, np.float32)[0]    # (K,6)
    theta = np.asarray(theta, np.float32)
    phi = np.asarray(phi, np.float32)
    res = int(res)
    H = W = res

    # ---- host prep: projection + per-group normalization (identical
    # arithmetic to the reference so the fp32 ball-query boundary decisions
    # match), then slab-filter points per (group, core)
    sint, cost = np.sin(theta), np.cos(theta)
    sinp, cosp = np.sin(phi), np.cos(phi)
    U = np.stack([-sint, cost, np.zeros_like(theta)], -1)
    V = np.stack([cost * sinp, sint * sinp, cosp], -1)
    basis = np.stack([U, V], -1).astype(np.float32)          # (M,3,2)
    center3 = np.stack([cost * cosp, sint * cosp, sinp], -1).astype(np.float32)
    coords_mv = np.einsum('mnd,mdk->mnk',
                          (xyz[None] - center3[:, None]).astype(np.float32),
                          basis).astype(np.float32)          # (M,N,2)
    valid = (np.all(xyz[None] <= boxes[:, None, 3:], -1)
             & np.all(xyz[None] >= boxes[:, None, :3], -1))  # (K,N)
    pts = np.sort(features, -1)[:, -2:].astype(np.float32)   # (N,2)
    dfull = (pts[:, 1] - pts[:, 0]).astype(np.float32)       # (N,)
    p2 = np.array([H, W], np.float32)

    A = np.empty((G, 4, N), np.float32)
    for k in range(K):
        vmask = valid[k]
        for m in range(M):
            c = coords_mv[m].copy()
            vc = c[vmask]
            cmin = vc.min(0)
            cmax = vc.max(0)
            ctr = ((cmax + cmin) / 2).astype(np.float32)
            scale = (np.maximum(cmax - cmin, np.float32(1e-5)) / 2).astype(np.float32)
            cn = (((c - ctr) / scale + np.float32(1.0)) * np.float32(0.8)
                  * p2 / 2 + np.float32(0.1) * p2).astype(np.float32)
            cn[~vmask] = 1e6
            g = k * M + m
            A[g, 0] = cn[:, 0]
            A[g, 1] = cn[:, 1]
            A[g, 2] = cn[:, 0] * cn[:, 0] + cn[:, 1] * cn[:, 1]
            A[g, 3] = 1.0

    # slab masks and per-group block caps (max over cores)
    lo = np.arange(NCORES, dtype=np.float32) * SROWS + SLAB_LO   # (NCORES,)
    hi = np.arange(NCORES, dtype=np.float32) * SROWS + SLAB_HI
    xg = A[:, 0, :]                                              # (G,N)
    masks = (xg[:, None, :] > lo[None, :, None]) & \
            (xg[:, None, :] < hi[None, :, None])                 # (G,NCORES,N)
    counts = masks.sum(-1)                                       # (G,NCORES)
    capblks = tuple(int(x) for x in
                    np.maximum(1, -(-counts.max(1) // 128)))     # ceil/128
    BTOT = int(sum(capblks))
    PTOT = 128 * BTOT

    if _CACHE.get("capblks") != capblks:
        _CACHE.clear()
        _CACHE["capblks"] = capblks
        _CACHE["nc"] = _build_nc(capblks)
        _CACHE["disp"] = _build_dispatch(_CACHE["nc"])

    gx, gy = np.meshgrid(np.arange(H), np.arange(W), indexing='ij')
    samples = np.stack([gx, gy], -1).reshape(-1, 2).astype(np.float32)  # (S,2)

    in_maps = []
    for cidx in range(NCORES):
        AXc = np.empty((4, PTOT), np.float32)
        AXc[0] = 1e6
        AXc[1] = 1e6
        AXc[2] = 2e12
        AXc[3] = 1.0
        PDc = np.zeros((128, 2 * BTOT), BF16)
        PDc[:, 1::2] = 1.0
        poff = 0
        boff = 0
        for g in range(G):
            nb = capblks[g]
            idx = np.nonzero(masks[g, cidx])[0]
            n = idx.size
            AXc[:, poff:poff + n] = A[g][:, idx]
            dpad = np.zeros(128 * nb, np.float32)
            dpad[:n] = dfull[idx]
            PDc[:, 2 * boff:2 * (boff + nb):2] = \
                dpad.reshape(nb, 128).T.astype(BF16)
            poff += 128 * nb
            boff += nb
        s = samples[cidx * SLOC:(cidx + 1) * SLOC]
        B4c = np.stack([
            2.0 * s[:, 0], 2.0 * s[:, 1],
            -np.ones(SLOC, np.float32),
            RADIUS2 - (s[:, 0] ** 2 + s[:, 1] ** 2),
        ]).astype(np.float32)
        TRIc = np.triu(np.ones((128, 128), np.float32), 1)
        np.fill_diagonal(TRIc, -float(NSAMPLE))
        FIXc = (np.ones((128, 128), np.float32) - TRIc)
        in_maps.append({"AX": AXc, "PD": PDc, "B4": B4c,
                        "TRIB": TRIc.astype(BF16), "FIXB": FIXc.astype(BF16)})

    results = _dispatch(in_maps)
    _last['exec_time_ns'] = None
    out_g = results["OUT"].reshape(NCORES, G, SROWS, W).astype(np.float32)
    full = np.concatenate([out_g[c] for c in range(NCORES)], axis=1)  # (G,H,W)
    out = np.broadcast_to(full[:, None, :, :], (G, 3, H, W)).astype(np.float32)
    return np.ascontiguousarray(out)


# revision 14
# speedup vs baseline: 10.0737x; 1.5413x over previous
import numpy as np
from contextlib import ExitStack

import concourse.bass as bass
import concourse.tile as tile
from concourse import bacc, mybir

# problem constants (hardcoded per contract)
N = 4096          # points
C = 20            # feature channels
K = 6             # boxes
M = 3             # views
G = K * M         # 18 groups
RES = 48          # H = W
NCORES = 8
SROWS = RES // NCORES          # 6 grid rows per core
SLOC = SROWS * RES             # 288 cells per core
NSAMPLE = 16
RADIUS2 = 9.0

TRACE = False
_last = {}

_f32 = mybir.dt.float32
_bf16 = mybir.dt.bfloat16
_ALU = mybir.AluOpType
_ACT = mybir.ActivationFunctionType
BF16 = mybir.dt.np(_bf16)

# x-slab half-width: a point can only be in-ball for a cell row gx when
# |x - gx| < RADIUS, so core c (rows 6c..6c+5) only needs x in (6c-3, 6c+8)
SLAB_LO = -3.0
SLAB_HI = float(SROWS) + 5.0


def _build_nc(capblks):
    """Per-group-capped ball query + first-16 aggregation.

    Inputs per core (slab-filtered points, group-major):
      AX   [3, 128*sum(capblks)] f32  rows [x, y, x^2+y^2] (pad: 1e6,1e6,2e12)
      PD   [128, sum(capblks)] bf16   per-block point scores s1-s0
      ONES [1, 128*max(capblks)] f32  constant-term row for the score matmul
      B4   [4, SLOC] f32  cell polynomials [2sx, 2sy, -1, R^2-sx^2-sy^2]
      TRIB/FIXB [128,128] bf16  prefix-scan / full-sum upgrade matrices
    Output: OUT [G, SLOC] bf16.
    """
    BTOT = int(sum(capblks))
    PTOT = 128 * BTOT
    maxnb = int(max(capblks))
    nc = bacc.Bacc("TRN2", target_bir_lowering=False, debug=False,
                   num_devices=NCORES)
    AX = nc.dram_tensor("AX", [3, PTOT], _f32, kind="ExternalInput").ap()
    PD = nc.dram_tensor("PD", [128, BTOT], _bf16, kind="ExternalInput").ap()
    ONES = nc.dram_tensor("ONES", [1, 128 * maxnb], _f32,
                          kind="ExternalInput").ap()
    B4 = nc.dram_tensor("B4", [4, SLOC], _f32, kind="ExternalInput").ap()
    TRIB = nc.dram_tensor("TRIB", [128, 128], _bf16, kind="ExternalInput").ap()
    FIXB = nc.dram_tensor("FIXB", [128, 128], _bf16, kind="ExternalInput").ap()
    OUT = nc.dram_tensor("OUT", [G, SLOC], _bf16, kind="ExternalOutput").ap()

    with ExitStack() as ctx:
        tc = ctx.enter_context(tile.TileContext(nc))
        consts = ctx.enter_context(tc.tile_pool(name="consts", bufs=1))
        apool = ctx.enter_context(tc.tile_pool(name="apool", bufs=2))
        ppool = ctx.enter_context(tc.tile_pool(name="ppool", bufs=2))
        wpool = ctx.enter_context(tc.tile_pool(name="wpool", bufs=4))
        spool = ctx.enter_context(tc.tile_pool(name="spool", bufs=4))
        fin = ctx.enter_context(tc.tile_pool(name="fin", bufs=1))
        rowpool = ctx.enter_context(tc.tile_pool(name="rowpool", bufs=2))
        scps = ctx.enter_context(
            tc.tile_pool(name="scps", bufs=3, space=bass.MemorySpace.PSUM))
        ups = ctx.enter_context(
            tc.tile_pool(name="ups", bufs=2, space=bass.MemorySpace.PSUM))
        stps = ctx.enter_context(
            tc.tile_pool(name="stps", bufs=2, space=bass.MemorySpace.PSUM))

        b4_t = consts.tile([4, SLOC], _f32)
        nc.sync.dma_start(b4_t[:], B4)
        tri_t = consts.tile([128, 128], _bf16)
        nc.sync.dma_start(tri_t[:], TRIB)
        fix_t = consts.tile([128, 128], _bf16)
        nc.sync.dma_start(fix_t[:], FIXB)

        d_all = fin.tile([G, SLOC], _f32, tag="d_all")
        c_all = fin.tile([G, SLOC], _f32, tag="c_all")

        poff = 0
        boff = 0
        for g in range(G):
            nb = int(capblks[g])
            a_t = apool.tile([4, 128 * nb], _f32, tag="a")
            nc.sync.dma_start(a_t[0:3, :], AX[:, poff:poff + 128 * nb])
            # compute engines need 32-aligned partition bases, so the
            # constant row is DMA'd from a device-cached DRAM tensor
            nc.sync.dma_start(a_t[3:4, :], ONES[:, :128 * nb])
            p_t = ppool.tile([128, 2 * nb], _bf16, tag="p")
            nc.sync.dma_start(p_t[:, 0::2], PD[:, boff:boff + nb])
            nc.vector.memset(p_t[:, 1::2], 1.0)
            u_ps = ups.tile([128, SLOC], _f32, tag="u")
            st_ps = stps.tile([2, SLOC], _f32, tag="st")
            for b in range(nb):
                score_ps = scps.tile([128, SLOC], _f32, tag="sc")
                nc.tensor.matmul(score_ps[:], a_t[:, 128 * b:128 * (b + 1)],
                                 b4_t[:], start=True, stop=True)
                within = wpool.tile([128, SLOC], _bf16, tag="w")
                nc.vector.tensor_scalar(within[:], score_ps[:], 0.0, None,
                                        _ALU.is_gt)
                # u = (total within count of prior blocks) + excl prefix
                #     - 16*within; sel = u < 0 picks the first 16 in-ball
                nc.tensor.matmul(u_ps[:], tri_t[:], within[:],
                                 start=(b == 0), stop=(b == nb - 1))
                sel = spool.tile([128, SLOC], _bf16, tag="s")
                nc.vector.tensor_scalar(sel[:], u_ps[:], 0.0, None, _ALU.is_lt)
                if b < nb - 1:
                    # upgrade this block's TRI contribution to its full
                    # within-count so u carries across blocks
                    nc.tensor.matmul(u_ps[:], fix_t[:], within[:],
                                     start=False, stop=False)
                # state rows: [sum(s1-s0), cnt] over selected points
                nc.tensor.matmul(st_ps[:], p_t[:, 2 * b:2 * (b + 1)], sel[:],
                                 start=(b == 0), stop=(b == nb - 1))
            # bounce the state rows through SBUF (ACT at partition 0), then
            # scatter to per-group partitions with DMA (no alignment limits)
            tmp = rowpool.tile([2, SLOC], _f32, tag="tmp")
            nc.scalar.activation(tmp[:], st_ps[:], _ACT.Copy)
            nc.sync.dma_start(d_all[g:g + 1, :], tmp[0:1, :])
            nc.sync.dma_start(c_all[g:g + 1, :], tmp[1:2, :])
            poff += 128 * nb
            boff += nb

        # finalize all groups at once:
        # out = (cnt>0) * sigmoid(sum(s1-s0)/max(cnt,1)) * 255
        cntc = fin.tile([G, SLOC], _f32, tag="cntc")
        nc.vector.tensor_scalar(cntc[:], c_all[:], 1.0, None, _ALU.max)
        rcp = fin.tile([G, SLOC], _f32, tag="rcp")
        nc.vector.reciprocal(rcp[:], cntc[:])
        nfd = fin.tile([G, SLOC], _f32, tag="nfd")
        nc.vector.tensor_tensor(nfd[:], d_all[:], rcp[:], _ALU.mult)
        sig = fin.tile([G, SLOC], _f32, tag="sig")
        nc.scalar.activation(sig[:], nfd[:], _ACT.Sigmoid)
        gate = fin.tile([G, SLOC], _f32, tag="gate")
        nc.vector.tensor_scalar(gate[:], c_all[:], 0.5, 255.0,
                                _ALU.is_gt, _ALU.mult)
        orow = fin.tile([G, SLOC], _bf16, tag="orow")
        nc.vector.tensor_tensor(orow[:], sig[:], gate[:], _ALU.mult)
        nc.sync.dma_start(OUT, orow[:])
    nc.compile()
    return nc


# ---------------------------------------------------------------------------
# Cached SPMD dispatch. run_bass_kernel_spmd rebuilds its jax.jit closure on
# every invocation, which forces a full XLA retrace+recompile (~0.9s) per
# call; the NEFF itself is unchanged between calls. Build the jitted
# shard_map executable once and reuse it, fetch the output with a single
# host transfer, and keep value-independent inputs resident on device.
# ---------------------------------------------------------------------------

_CACHE = {}
_CONST_NAMES = {"ONES", "B4", "TRIB", "FIXB"}


def _build_dispatch(nc):
    import jax
    from jax.experimental.shard_map import shard_map
    from jax.sharding import Mesh, NamedSharding, PartitionSpec
    from concourse.bass2jax import (
        _bass_exec_p, install_neuronx_cc_hook, partition_id_tensor)

    install_neuronx_cc_hook()
    assert nc.dbg_addr is None

    partition_name = nc.partition_id_tensor.name if nc.partition_id_tensor else None
    in_names, out_names, out_avals, zero_shapes = [], [], [], []
    for alloc in nc.m.functions[0].allocations:
        if not isinstance(alloc, mybir.MemoryLocationSet):
            continue
        name = alloc.memorylocations[0].name
        if alloc.kind == "ExternalInput":
            if name != partition_name:
                in_names.append(name)
        elif alloc.kind == "ExternalOutput":
            shape = tuple(alloc.tensor_shape)
            dtype = mybir.dt.np(alloc.dtype)
            out_names.append(name)
            out_avals.append(jax.core.ShapedArray(shape, dtype))
            zero_shapes.append((shape, dtype))
    n_params = len(in_names)
    bind_names = in_names + out_names
    if partition_name is not None:
        bind_names.append(partition_name)

    def _body(*args):
        operands = list(args)
        if partition_name is not None:
            operands.append(partition_id_tensor())
        outs = _bass_exec_p.bind(
            *operands,
            out_avals=tuple(out_avals),
            in_names=tuple(bind_names),
            out_names=tuple(out_names),
            lowering_input_output_aliases=(),
            sim_require_finite=True,
            sim_require_nnan=True,
            nc=nc,
        )
        return tuple(outs)

    devices = jax.devices()[:NCORES]
    assert len(devices) == NCORES
    mesh = Mesh(np.asarray(devices), ("core",))
    n_outs = len(out_names)
    donate = tuple(range(n_params, n_params + n_outs))
    in_specs = (PartitionSpec("core"),) * (n_params + n_outs)
    out_specs = (PartitionSpec("core"),) * n_outs
    sharded = jax.jit(
        shard_map(_body, mesh=mesh, in_specs=in_specs, out_specs=out_specs,
                  check_rep=False),
        donate_argnums=donate,
        keep_unused=True,
    )
    sharding = NamedSharding(mesh, PartitionSpec("core"))

    def put(x):
        return jax.device_put(x, sharding)

    return {
        "fn": sharded, "in_names": in_names, "out_names": out_names,
        "zero_shapes": zero_shapes, "put": put,
    }


def _dispatch(in_maps, const_map):
    """in_maps: per-core dict of value-dependent arrays. const_map: dict of
    per-core-stacked value-independent arrays, committed to device once."""
    d = _CACHE["disp"]
    if "const_cache" not in d:
        d["const_cache"] = {name: d["put"](arr)
                            for name, arr in const_map.items()}
    concat_in = []
    for name in d["in_names"]:
        if name in d["const_cache"]:
            concat_in.append(d["const_cache"][name])
            continue
        arr = np.concatenate([m[name] for m in in_maps], axis=0)
        concat_in.append(arr)
    concat_zeros = [np.zeros((NCORES * s[0], *s[1:]), dt)
                    for s, dt in d["zero_shapes"]]
    out_arrs = d["fn"](*concat_in, *concat_zeros)
    return {name: np.asarray(out_arrs[i]) for i, name in enumerate(d["out_names"])}


def _build_consts(capblks):
    maxnb = int(max(capblks))
    gx, gy = np.meshgrid(np.arange(RES), np.arange(RES), indexing='ij')
    samples = np.stack([gx, gy], -1).reshape(-1, 2).astype(np.float32)
    TRIc = np.triu(np.ones((128, 128), np.float32), 1)
    np.fill_diagonal(TRIc, -float(NSAMPLE))
    FIXc = np.ones((128, 128), np.float32) - TRIc
    onesr = np.ones((1, 128 * maxnb), np.float32)
    b4s, oness, tris, fixs = [], [], [], []
    for cidx in range(NCORES):
        s = samples[cidx * SLOC:(cidx + 1) * SLOC]
        b4s.append(np.stack([
            2.0 * s[:, 0], 2.0 * s[:, 1],
            -np.ones(SLOC, np.float32),
            RADIUS2 - (s[:, 0] ** 2 + s[:, 1] ** 2),
        ]).astype(np.float32))
        oness.append(onesr)
        tris.append(TRIc.astype(BF16))
        fixs.append(FIXc.astype(BF16))
    return {
        "B4": np.concatenate(b4s, axis=0),
        "ONES": np.concatenate(oness, axis=0),
        "TRIB": np.concatenate(tris, axis=0),
        "FIXB": np.concatenate(fixs, axis=0),
    }


def kernel(xyz, features, boxes, theta, phi, res):
    xyz = np.asarray(xyz, np.float32)[0]        # (N,3)
    features = np.asarray(features, np.float32)[0]  # (N,C)
    boxes = np.asarray(boxes, np.float32)[0]    # (K,6)
    theta = np.asarray(theta, np.float32)
    phi = np.asarray(phi, np.float32)
    res = int(res)
    H = W = res

    # ---- host prep: projection + per-group normalization (identical
    # arithmetic to the reference so the fp32 ball-query boundary decisions
    # match), then slab-filter points per (group, core)
    sint, cost = np.sin(theta), np.cos(theta)
    sinp, cosp = np.sin(phi), np.cos(phi)
    U = np.stack([-sint, cost, np.zeros_like(theta)], -1)
    V = np.stack([cost * sinp, sint * sinp, cosp], -1)
    basis = np.stack([U, V], -1).astype(np.float32)          # (M,3,2)
    center3 = np.stack([cost * cosp, sint * cosp, sinp], -1).astype(np.float32)
    coords_mv = np.einsum('mnd,mdk->mnk',
                          (xyz[None] - center3[:, None]).astype(np.float32),
                          basis).astype(np.float32)          # (M,N,2)
    valid = (np.all(xyz[None] <= boxes[:, None, 3:], -1)
             & np.all(xyz[None] >= boxes[:, None, :3], -1))  # (K,N)
    pts = np.sort(features, -1)[:, -2:].astype(np.float32)   # (N,2)
    dfull = (pts[:, 1] - pts[:, 0]).astype(np.float32)       # (N,)
    p2 = np.array([H, W], np.float32)

    # vectorized per-(box,view) normalization; min/max over the valid subset
    # equals the masked min/max exactly, and the elementwise chain below is
    # the same fp32 op sequence as the reference
    vm4 = valid[:, None, :, None]                            # (K,1,N,1)
    cm = np.broadcast_to(coords_mv[None], (K, M, N, 2))
    cmax = np.where(vm4, cm, -np.inf).max(2)                 # (K,M,2)
    cmin = np.where(vm4, cm, np.inf).min(2)
    ctr = ((cmax + cmin) / 2).astype(np.float32)
    scale = (np.maximum(cmax - cmin, np.float32(1e-5)) / 2).astype(np.float32)
    cn = (((cm - ctr[:, :, None]) / scale[:, :, None] + np.float32(1.0))
          * np.float32(0.8) * p2 / 2 + np.float32(0.1) * p2).astype(np.float32)
    cn = np.where(vm4, cn, np.float32(1e6)).reshape(G, N, 2)
    A = np.empty((G, 3, N), np.float32)
    A[:, 0] = cn[..., 0]
    A[:, 1] = cn[..., 1]
    A[:, 2] = cn[..., 0] * cn[..., 0] + cn[..., 1] * cn[..., 1]

    # slab masks and per-group block caps (max over cores)
    lo = np.arange(NCORES, dtype=np.float32) * SROWS + SLAB_LO   # (NCORES,)
    hi = np.arange(NCORES, dtype=np.float32) * SROWS + SLAB_HI
    xg = A[:, 0, :]                                              # (G,N)
    masks = (xg[:, None, :] > lo[None, :, None]) & \
            (xg[:, None, :] < hi[None, :, None])                 # (G,NCORES,N)
    counts = masks.sum(-1)                                       # (G,NCORES)
    capblks = tuple(int(x) for x in
                    np.maximum(1, -(-counts.max(1) // 128)))     # ceil/128
    BTOT = int(sum(capblks))
    PTOT = 128 * BTOT
    poffs = np.concatenate([[0], np.cumsum([128 * b for b in capblks])])
    boffs = np.concatenate([[0], np.cumsum(capblks)])

    if _CACHE.get("capblks") != capblks:
        _CACHE.clear()
        _CACHE["capblks"] = capblks
        _CACHE["nc"] = _build_nc(capblks)
        _CACHE["disp"] = _build_dispatch(_CACHE["nc"])
        _CACHE["consts"] = _build_consts(capblks)

    Aflat = np.ascontiguousarray(A.transpose(1, 0, 2)).reshape(3, G * N)
    in_maps = []
    for cidx in range(NCORES):
        AXc = np.empty((3, PTOT), np.float32)
        AXc[0] = 1e6
        AXc[1] = 1e6
        AXc[2] = 2e12
        PDc = np.zeros((128, BTOT), BF16)
        idxs = [np.nonzero(masks[g, cidx])[0] for g in range(G)]
        src = np.concatenate([g * N + idxs[g] for g in range(G)])
        q = np.concatenate([np.arange(idxs[g].size) for g in range(G)])
        gid = np.concatenate([np.full(idxs[g].size, g) for g in range(G)])
        dst = poffs[gid] + q
        AXc[:, dst] = Aflat[:, src]
        PDc[q % 128, boffs[gid] + q // 128] = dfull[src % N]
        in_maps.append({"AX": AXc, "PD": PDc})

    results = _dispatch(in_maps, _CACHE["consts"])
    _last['exec_time_ns'] = None
    out_g = results["OUT"].reshape(NCORES, G, SROWS, W).astype(np.float32)
    full = np.concatenate([out_g[c] for c in range(NCORES)], axis=1)  # (G,H,W)
    out = np.broadcast_to(full[:, None, :, :], (G, 3, H, W)).astype(np.float32)
    return np.ascontiguousarray(out)
